# revision 1
# baseline (speedup 1.0000x reference)
"""ConvSelfAttention Trainium2 kernel.

Reference computation (B=4, C=512, N=2048, H=8 heads, D=64):
    qkv = w_qkv @ x          (pointwise conv == matmul over channels)
    per head: sim = (q*D^-.5)^T k ; attn = softmax(sim, axis=j)
    out = attn @ v^T ; y = w_out @ out_heads + b_out

Sharding: 8 cores = 4 batches x 2 head-groups (4 heads each). Each core
computes its batch's x-projections restricted to its 4 heads, runs
attention, and produces a partial output projection y_part[c, n]
(sum over its heads' hd columns of w_out). Host sums the two partials
per batch and adds the bias.

On-chip layout notes:
  - All matmuls take bf16 inputs (fp32 PSUM accumulation); fp32 matmul
    runs at half rate on the PE.
  - Attention is computed transposed: S^T[j, i] = k^T q so softmax's
    sum over j is a matmul contraction. exp() runs on ScalarE from
    PSUM in wide [128, 3*512] instructions. The softmax denominator
    comes from a ones-column appended to v^T (PV matmul row 64).
  - Normalization 1/l is broadcast across partitions with a rank-1
    matmul (ones ⊗ recip_l) since engines cannot partition-broadcast.
  - Heads are processed in pairs living in SBUF partitions 0-63/64-127
    so the K=64 S^T matmuls auto-derive tile_position (0,0)/(64,0) and
    run concurrently in the two 64-row PE tiles.
"""

import numpy as np
import ml_dtypes

B, C, N = 4, 512, 2048
H, D = 8, 64
HID = H * D
SCALE = D ** -0.5
NCORES = 8
HPC = 4          # heads per core
NT = 4           # i-tiles of 512
KT = 4           # k-tiles of 128 over C
JB = 16          # j-blocks of 128
SLOT = 512
SPW = 2          # S^T psum slots per tile (psum banks)

bf16 = ml_dtypes.bfloat16

_PROG = None


def _build_program(reps=1, skip=()):
    import concourse.mybir as mybir
    import concourse.tile as tile
    from concourse import bacc

    fp32 = mybir.dt.float32
    bfl = mybir.dt.bfloat16
    Exp = mybir.ActivationFunctionType.Exp

    nc = bacc.Bacc("TRN2", target_bir_lowering=False, debug=False)

    x_d = nc.dram_tensor("x", [C, N], bfl, kind="ExternalInput")
    wqk_d = nc.dram_tensor("wqk", [C, 512], bfl, kind="ExternalInput")
    wv_d = nc.dram_tensor("wv", [C, 256], bfl, kind="ExternalInput")
    wo_d = nc.dram_tensor("wo", [256, C], bfl, kind="ExternalInput")
    y_d = nc.dram_tensor("y", [C, N], fp32, kind="ExternalOutput")

    import contextlib

    with tile.TileContext(nc) as tc:
        loop_cm = tc.For_i(0, reps, 1) if reps > 1 else contextlib.nullcontext()
        with (
            loop_cm,
            tc.tile_pool(name="const", bufs=1) as constp,
            tc.tile_pool(name="big", bufs=1) as bigp,
            tc.tile_pool(name="pt", bufs=34) as ptp,
            tc.tile_pool(name="ov", bufs=4) as ovp,
            tc.tile_pool(name="spsum", bufs=2, space="PSUM") as sp,
            tc.tile_pool(name="wpsum", bufs=4, space="PSUM") as wp,
        ):
            # ---- constants
            # ones2: block "selector" for the merged 1/l broadcast matmul:
            # lb = ones2.T @ lrow puts lrow row0 on partitions 0-63 and row1
            # on partitions 64-127.
            ones2 = constp.tile([128, 128], fp32, tag="ones", name="ones")
            nc.vector.memset(ones2[:], 0.0)
            nc.vector.memset(ones2[0:1, 0:64], 1.0)
            nc.vector.memset(ones2[64:65, 64:128], 1.0)
            lrow = constp.tile([128, SLOT], fp32, tag="lrow", name="lrow")
            nc.vector.memset(lrow[:], 0.0)

            # ---- input loads. Order matters for pipeline startup: the qk
            # projection's first psum group reads wqk[kt][:, 0:128] and
            # x[kt][:, 0:512] for all kt, so load weights first and x in
            # column chunks, chunk-major (subtile deps let the first matmuls
            # start after ~400KB instead of the full 3MB).
            wqk_sb = []
            for kt in range(KT):
                t = bigp.tile([128, 512], bfl, tag=f"wqk{kt}", name=f"wqk{kt}")
                nc.sync.dma_start(t[:], wqk_d[kt * 128:(kt + 1) * 128, :])
                wqk_sb.append(t)
            x_sb = [
                bigp.tile([128, N], bfl, tag=f"x{kt}", name=f"x{kt}")
                for kt in range(KT)
            ]
            for nt in range(NT):
                for kt in range(KT):
                    nc.sync.dma_start(
                        x_sb[kt][:, nt * SLOT:(nt + 1) * SLOT],
                        x_d[kt * 128:(kt + 1) * 128, nt * SLOT:(nt + 1) * SLOT],
                    )
            wv_sb = []
            for kt in range(KT):
                t = bigp.tile([128, 256], bfl, tag=f"wv{kt}", name=f"wv{kt}")
                nc.sync.dma_start(t[:], wv_d[kt * 128:(kt + 1) * 128, :])
                wv_sb.append(t)
            wo_sb = []
            for kt in range(2):
                t = bigp.tile([128, 512], bfl, tag=f"wo{kt}", name=f"wo{kt}")
                nc.sync.dma_start(t[:], wo_d[kt * 128:(kt + 1) * 128, :])
                wo_sb.append(t)

            # ---- QK projection -> q_sb[hp], k_sb[hp] (2 heads stacked per tile)
            # host column order: q-hp0 | k-hp0 | q-hp1 | k-hp1 so that hp0's
            # attention can start after only half the projection.
            q_sb = [bigp.tile([128, N], bfl, tag=f"q{hp}", name=f"q{hp}") for hp in range(2)]
            k_sb = [bigp.tile([128, N], bfl, tag=f"k{hp}", name=f"k{hp}") for hp in range(2)]
            dest = [q_sb[0], k_sb[0], q_sb[1], k_sb[1]]

            def emit_qkproj(mt, nts=range(NT)):
                for nt in nts:
                    ps = wp.tile([128, SLOT], fp32, tag="w", name="wps")
                    for kt in range(KT):
                        nc.tensor.matmul(
                            ps[:],
                            wqk_sb[kt][:, mt * 128:(mt + 1) * 128],
                            x_sb[kt][:, nt * SLOT:(nt + 1) * SLOT],
                            start=(kt == 0),
                            stop=(kt == KT - 1),
                        )
                    nc.vector.tensor_copy(
                        dest[mt][:, nt * SLOT:(nt + 1) * SLOT], ps[:]
                    )

            # ---- v^T projection, augmented with a ones column per head
            # vt_sb[jb] : [128 (j), HPC, 65] ; [:, h, 0:64] = v^T, [:, h, 64] = 1
            vt_sb = []

            def emit_vtproj():
                for jb in range(JB):
                    t = bigp.tile([128, HPC, 65], bfl, tag=f"vt{jb}", name=f"vt{jb}")
                    nc.vector.memset(t[:, :, 64:65], 1.0)
                    ps = wp.tile([128, SLOT], fp32, tag="w", name="wps")
                    for kt in range(KT):
                        nc.tensor.matmul(
                            ps[:, 0:256],
                            x_sb[kt][:, jb * 128:(jb + 1) * 128],
                            wv_sb[kt][:],
                            start=(kt == 0),
                            stop=(kt == KT - 1),
                        )
                    nc.vector.tensor_copy(
                        t[:, :, 0:64],
                        ps[:, 0:256].rearrange("p (h d) -> p h d", h=HPC),
                    )
                    vt_sb.append(t)

            # ---- attention, software-pipelined per (it, hp) group
            oh_sb = [bigp.tile([128, N], bfl, tag=f"oh{hp}", name=f"oh{hp}") for hp in range(2)]
            groups = [(it, hp) for it in range(NT) for hp in range(2)]

            def emit_sim_exp(it, hp):
                """S^T matmuls + exp for one (i-tile, head-pair). Returns the
                list of (pt_tile, [(slot_idx, hh, jb), ...])."""
                out = []
                slots = [(jb, hh) for jb in range(JB) for hh in range(2)]
                for base in range(0, len(slots), SPW):
                    chunk = slots[base:base + SPW]
                    st = sp.tile([128, SPW * SLOT], fp32, tag="s", name="st")
                    for si, (jb, hh) in enumerate(chunk):
                        lo, hi = hh * 64, hh * 64 + 64
                        nc.tensor.matmul(
                            st[:, si * SLOT:(si + 1) * SLOT],
                            k_sb[hp][lo:hi, jb * 128:(jb + 1) * 128],
                            q_sb[hp][lo:hi, it * SLOT:(it + 1) * SLOT],
                        )
                    w = len(chunk) * SLOT
                    pt = ptp.tile([128, SPW * SLOT], bfl, tag="pt", name="pt")
                    if "exp" not in skip:
                        nc.scalar.activation(pt[:, 0:w], st[:, 0:w], Exp)
                    else:
                        # ablation: token ACT write so the tile is not unwritten
                        nc.scalar.activation(pt[:, 0:4], st[:, 0:4], Exp)
                    out.append((pt, chunk))
                return out

            def emit_pv_tail(it, hp, ptiles):
                if "pv" in skip:
                    return
                pv = [wp.tile([128, SLOT], fp32, tag="w", name="pv") for _ in range(2)]
                nmm = [0, 0]
                for pt, chunk in ptiles:
                    for si, (jb, hh) in enumerate(chunk):
                        nc.tensor.matmul(
                            pv[hh][0:65, :],
                            vt_sb[jb][:, hp * 2 + hh, :],
                            pt[:, si * SLOT:(si + 1) * SLOT],
                            start=(nmm[hh] == 0),
                            stop=(nmm[hh] == JB - 1),
                        )
                        nmm[hh] += 1
                ovs = []
                for hh in range(2):
                    ov = ovp.tile([65, SLOT], fp32, tag="ov", name="ov")
                    nc.vector.tensor_copy(ov[:], pv[hh][0:65, :])
                    nc.vector.reciprocal(lrow[hh * 64:hh * 64 + 1, :], ov[64:65, :])
                    ovs.append(ov)
                # single broadcast matmul: lb rows 0-63 = 1/l_h0, 64-127 = 1/l_h1
                lb = wp.tile([128, SLOT], fp32, tag="w", name="lb")
                nc.tensor.matmul(lb[:], ones2[:], lrow[:])
                for hh in range(2):
                    nc.vector.tensor_mul(
                        oh_sb[hp][hh * 64:(hh + 1) * 64,
                                  it * SLOT:(it + 1) * SLOT],
                        ovs[hh][0:64, :],
                        lb[hh * 64:(hh + 1) * 64, :],
                    )

            def emit_outproj(it):
                if "pv" in skip:
                    return
                for mt in range(4):
                    yp = wp.tile([128, SLOT], fp32, tag="w", name="yp")
                    for kt in range(2):
                        nc.tensor.matmul(
                            yp[:],
                            wo_sb[kt][:, mt * 128:(mt + 1) * 128],
                            oh_sb[kt][:, it * SLOT:(it + 1) * SLOT],
                            start=(kt == 0),
                            stop=(kt == 1),
                        )
                    ys = ovp.tile([128, SLOT], fp32, tag="ys", name="ys")
                    nc.vector.tensor_copy(ys[:], yp[:])
                    nc.sync.dma_start(
                        y_d[mt * 128:(mt + 1) * 128, it * SLOT:(it + 1) * SLOT],
                        ys[:],
                    )

            # Early start: project only what the first attention group needs
            # (q-hp0 i-tile 0 + all of k-hp0), launch it so ACT gets exp work
            # ASAP, then fill in the remaining projections while ACT chews on
            # group 0.
            emit_qkproj(0, nts=[0])
            emit_qkproj(1)
            first = emit_sim_exp(*groups[0])
            emit_qkproj(0, nts=[1, 2, 3])
            emit_qkproj(2)
            emit_qkproj(3)
            emit_vtproj()

            pending = (groups[0][0], groups[0][1], first)
            for it, hp in groups[1:]:
                ptiles = emit_sim_exp(it, hp)
                pit, php, pp = pending
                emit_pv_tail(pit, php, pp)
                if php == 1:
                    emit_outproj(pit)
                pending = (it, hp, ptiles)
            pit, php, pp = pending
            emit_pv_tail(pit, php, pp)
            emit_outproj(pit)

    nc.compile()
    return nc


def _get_program():
    global _PROG
    if _PROG is None:
        _PROG = _build_program()
    return _PROG


def _make_in_maps(x, w_qkv, w_out):
    xs = x.astype(bf16)
    in_maps = []
    for core in range(NCORES):
        b, g = divmod(core, 2)
        r0 = 256 * g
        wq = w_qkv[r0:r0 + 256, :] * SCALE
        wk = w_qkv[512 + r0:512 + r0 + 256, :]
        wv = w_qkv[1024 + r0:1024 + r0 + 256, :]
        wqkT = np.ascontiguousarray(
            np.concatenate(
                [wq[0:128], wk[0:128], wq[128:256], wk[128:256]], axis=0
            ).T.astype(bf16)
        )
        wvT = np.ascontiguousarray(wv.T.astype(bf16))
        woT = np.ascontiguousarray(w_out[:, r0:r0 + 256].T.astype(bf16))
        in_maps.append({
            "x": np.ascontiguousarray(xs[b]),
            "wqk": wqkT,
            "wv": wvT,
            "wo": woT,
        })
    return in_maps


def _gather(results, b_out):
    parts = [results[c]["y"] for c in range(NCORES)]
    y = np.stack([parts[2 * b] + parts[2 * b + 1] for b in range(B)])
    y += b_out[None, :, None]
    return y.astype(np.float32)


def kernel(x, w_qkv, w_out, b_out):
    from concourse.bass_utils import run_bass_kernel_spmd

    nc = _get_program()

    x = np.asarray(x, dtype=np.float32)
    w_qkv = np.asarray(w_qkv, dtype=np.float32)
    w_out = np.asarray(w_out, dtype=np.float32)
    b_out = np.asarray(b_out, dtype=np.float32)

    in_maps = _make_in_maps(x, w_qkv, w_out)
    res = run_bass_kernel_spmd(nc, in_maps, core_ids=list(range(NCORES)))
    return _gather(res.results, b_out)



# revision 41
# speedup vs baseline: 1.2641x; 1.2641x over previous
"""ConvSelfAttention Trainium2 kernel.

Reference computation (B=4, C=512, N=2048, H=8 heads, D=64):
    qkv = w_qkv @ x          (pointwise conv == matmul over channels)
    per head: sim = (q*D^-.5)^T k ; attn = softmax(sim, axis=j)
    out = attn @ v^T ; y = w_out @ out_heads + b_out

Sharding: 8 cores = 4 batches x 2 head-groups (4 heads each). Each core
computes its batch's x-projections restricted to its 4 heads, runs
attention, and produces a partial output projection y_part[c, n]
(sum over its heads' hd columns of w_out). Host sums the two partials
per batch and adds the bias.

Pipeline: the kernel is PE-bound, so every other engine's latency must
hide behind PE work. Each steady-state iteration WEAVES the previous
group's P@V matmuls (65-wide, ~27ns each) between the next group's
S^T matmuls so the S^T psum double-buffer rotation (which waits on
exp) never stalls PE: per j-block the PE stream is [2 sim matmuls,
8 P@V matmuls] and the exp of block j must only complete before block
j+2's sim matmuls — about two weave slots of slack.

On-chip layout notes:
  - All matmuls take bf16 inputs (fp32 PSUM accumulation).
  - Attention is computed transposed: S^T[j, i] = k^T q so softmax's
    sum over j is a matmul contraction (ones column appended to v^T).
  - exp() is split ACT/DVE: ACT runs true Exp; DVE runs a Schraudolph
    bit-trick (int16 = s*log2e*128 + const, bitcast bf16 ~= exp(s),
    ~2-3% ripple, bias cancels in softmax) via one tensor_scalar.
    Measured end-to-end rel err ~1e-2 vs the 2e-2 budget.
  - P@V runs in the [i, d] orientation (pt as stationary operand) so
    each matmul covers 128 output partitions at free dim 65; this
    halves PE time vs the [d, i] orientation (65 output partitions).
    Normalization happens with i on partitions (per-partition scalar
    multiply on GPSIMD), then a PE transpose restores [hd, n] for the
    output projection.
  - Heads are processed in pairs on SBUF partitions 0-63/64-127 so the
    K=64 S^T matmuls auto-derive tile_position (0,0)/(64,0).
"""

import numpy as np
import ml_dtypes

B, C, N = 4, 512, 2048
H, D = 8, 64
HID = H * D
SCALE = D ** -0.5
NCORES = 8
HPC = 4          # heads per core
NT = 4           # i-tiles of 512
KT = 4           # k-tiles of 128 over C
JB = 16          # j-blocks of 128
SLOT = 512
SPW = 2          # S^T psum slots per tile (psum banks)

# exp engine per (j-block, head) unit of [128, 512]: "A"=ACT exact exp,
# "D"=DVE schraudolph. 17/15, spread so no engine clusters (the s-psum
# slot a unit frees is needed 4 units later by the next sim matmul).
# GPSIMD cannot touch PSUM on real hardware, so it gets no exp units.
EXP_ENG = ["A", "D", "A", "D", "A", "D", "A", "D",
           "A", "D", "A", "D", "A", "D", "A", "A",
           "D", "A", "D", "A", "D", "A", "D", "A",
           "D", "A", "A", "D", "A", "D", "A", "D"]

LOG2E = 1.4426950408889634
SCH_A = 128.0 * LOG2E
SCH_B = 127.0 * 128.0 - 4.0

bf16 = ml_dtypes.bfloat16

_PROG = None


def _build_program(reps=1, skip=()):
    import concourse.mybir as mybir
    import concourse.tile as tile
    from concourse import bacc

    fp32 = mybir.dt.float32
    bfl = mybir.dt.bfloat16
    i16 = mybir.dt.int16
    Exp = mybir.ActivationFunctionType.Exp
    Mult = mybir.AluOpType.mult

    nc = bacc.Bacc("TRN2", target_bir_lowering=False, debug=False)

    x_d = nc.dram_tensor("x", [C, N], bfl, kind="ExternalInput")
    wqk_d = nc.dram_tensor("wqk", [C, 512], bfl, kind="ExternalInput")
    wv_d = nc.dram_tensor("wv", [C, 256], bfl, kind="ExternalInput")
    wo_d = nc.dram_tensor("wo", [256, C], bfl, kind="ExternalInput")
    id_d = nc.dram_tensor("ident", [128, 128], bfl, kind="ExternalInput")
    y_d = nc.dram_tensor("y", [C, N], bfl, kind="ExternalOutput")

    import contextlib

    with tile.TileContext(nc) as tc:
        loop_cm = tc.For_i(0, reps, 1) if reps > 1 else contextlib.nullcontext()
        with (
            loop_cm,
            tc.tile_pool(name="const", bufs=1) as constp,
            tc.tile_pool(name="big", bufs=1) as bigp,
            tc.tile_pool(name="pt", bufs=68) as ptp,
            tc.tile_pool(name="ov", bufs=8) as ovp,
            tc.tile_pool(name="spsum", bufs=4, space="PSUM") as sp,
            tc.tile_pool(name="wpsum", bufs=2, space="PSUM") as wp,
            tc.tile_pool(name="popsum", bufs=2, space="PSUM") as pop,
        ):
            # ---- input loads, consolidated: one SBUF tile per tensor with
            # the C-tiles stacked on a middle dim, few DMA instructions
            # (HWDGE descriptor-gen is ~650ns per DMA and serializes
            # startup), ordered so the first projection's operands (wqk
            # mt0-1 columns + x nt0) land first. ident is only needed at
            # the first transpose ~25us in, so it loads last.
            wqk_sb = bigp.tile([128, KT, 512], bfl, tag="wqk", name="wqk")
            wqk_r = wqk_d.rearrange("(kt p) m -> p kt m", kt=KT)
            nc.sync.dma_start(wqk_sb[:, :, 0:256], wqk_r[:, :, 0:256])
            x_sb = bigp.tile([128, KT, N], bfl, tag="x", name="x")
            x_r = x_d.rearrange("(kt p) n -> p kt n", kt=KT)

            def load_x(nt):
                nc.sync.dma_start(
                    x_sb[:, :, nt * SLOT:(nt + 1) * SLOT],
                    x_r[:, :, nt * SLOT:(nt + 1) * SLOT],
                )

            load_x(0)
            load_x(1)
            nc.sync.dma_start(wqk_sb[:, :, 256:512], wqk_r[:, :, 256:512])
            load_x(2)
            load_x(3)
            wv_sb = bigp.tile([128, KT, 256], bfl, tag="wv", name="wv")
            nc.sync.dma_start(
                wv_sb[:], wv_d.rearrange("(kt p) m -> p kt m", kt=KT)
            )
            wo_sb = bigp.tile([128, 2, 512], bfl, tag="wo", name="wo")
            nc.sync.dma_start(
                wo_sb[:], wo_d.rearrange("(kt p) m -> p kt m", kt=2)
            )
            ident = constp.tile([128, 128], bfl, tag="id", name="ident")
            nc.sync.dma_start(ident[:], id_d[:, :])

            # ---- QK projection -> q_sb[hp], k_sb[hp] (2 heads stacked)
            # host column order: q-hp0 | k-hp0 | q-hp1 | k-hp1.
            q_sb = [bigp.tile([128, N], bfl, tag=f"q{hp}", name=f"q{hp}") for hp in range(2)]
            k_sb = [bigp.tile([128, N], bfl, tag=f"k{hp}", name=f"k{hp}") for hp in range(2)]
            dest = [q_sb[0], k_sb[0], q_sb[1], k_sb[1]]

            def emit_qkproj_tile(mt, nt):
                ps = wp.tile([128, SLOT], fp32, tag="w", name="wps")
                for kt in range(KT):
                    nc.tensor.matmul(
                        ps[:],
                        wqk_sb[:, kt, mt * 128:(mt + 1) * 128],
                        x_sb[:, kt, nt * SLOT:(nt + 1) * SLOT],
                        start=(kt == 0),
                        stop=(kt == KT - 1),
                    )
                if (mt + nt) % 2 == 0:
                    nc.vector.tensor_copy(
                        dest[mt][:, nt * SLOT:(nt + 1) * SLOT], ps[:]
                    )
                else:
                    nc.scalar.copy(dest[mt][:, nt * SLOT:(nt + 1) * SLOT],
                                   ps[:])

            # ---- v^T projection, augmented with a ones column per head
            # vt_sb[jb] : [128 (j), HPC, 65] ; [:, h, 0:64] = v^T, [:, h, 64]=1
            vt_sb = [None] * JB

            def emit_vtproj_tile(jb):
                t = bigp.tile([128, HPC, 65], bfl, tag=f"vt{jb}", name=f"vt{jb}")
                nc.vector.memset(t[:, :, 64:65], 1.0)
                ps = wp.tile([128, SLOT], fp32, tag="w", name="wps")
                for kt in range(KT):
                    nc.tensor.matmul(
                        ps[:, 0:256],
                        x_sb[:, kt, jb * 128:(jb + 1) * 128],
                        wv_sb[:, kt, :],
                        start=(kt == 0),
                        stop=(kt == KT - 1),
                    )
                if jb % 2 == 0:
                    nc.vector.tensor_copy(
                        t[:, :, 0:64],
                        ps[:, 0:256].rearrange("p (h d) -> p h d", h=HPC),
                    )
                else:
                    nc.scalar.copy(
                        t[:, :, 0:64],
                        ps[:, 0:256].rearrange("p (h d) -> p h d", h=HPC),
                    )
                vt_sb[jb] = t

            # ---- attention
            oh_sb = [bigp.tile([128, N], bfl, tag=f"oh{hp}", name=f"oh{hp}") for hp in range(2)]
            groups = [(it, hp) for it in range(NT) for hp in range(2)]

            def emit_sim_unit(it, hp, u):
                """S^T matmul + exp for one (j-block, head) unit u=2*jb+hh.
                Returns the pt tile [128 j, 512 i] bf16."""
                jb, hh = divmod(u, 2)
                st = sp.tile([128, SLOT], fp32, tag="s", name="st")
                lo, hi = hh * 64, hh * 64 + 64
                nc.tensor.matmul(
                    st[:],
                    k_sb[hp][lo:hi, jb * 128:(jb + 1) * 128],
                    q_sb[hp][lo:hi, it * SLOT:(it + 1) * SLOT],
                )
                pt = ptp.tile([128, SLOT], bfl, tag="pt", name="pt")
                if "exp" not in skip:
                    if EXP_ENG[u] == "A":
                        nc.scalar.activation(pt[:], st[:], Exp)
                    else:
                        nc.vector.tensor_scalar(
                            pt[:].bitcast(i16), st[:],
                            SCH_A, SCH_B, Mult, mybir.AluOpType.add,
                        )
                else:
                    nc.scalar.activation(pt[:, 0:4], st[:, 0:4], Exp)
                return pt

            def make_pv_fillers(it, hp, ptiles):
                """(16 filler callables, state) for the weave: 8 matmul-chunks
                per po-half (each chunk = 2 j-blocks x 4 accumulations), with
                the normalize chain emitted as soon as its po half completes.
                state["ohis"] collects the normalized [i, hd] tiles for the
                transpose pass."""
                state = {"ohis": []}
                if "pv" in skip:
                    return [(lambda: None)] * JB, state

                def chunk(half, jc):
                    def f():
                        if jc == 0:
                            # Four accumulation groups share this psum bank,
                            # and a matmul's start_tensor_calc zeroes the
                            # whole bank (ZERO_REGION granularity), so the
                            # groups must accumulate onto an explicit memset
                            # with start=False throughout.
                            state[half] = pop.tile(
                                [128, 260], fp32, tag="po", name="po"
                            )
                            if half == 0:
                                nc.vector.memset(state[half][:], 0.0)
                            else:
                                nc.scalar.memzero(state[half][:])
                        po = state[half]
                        for jb in (2 * jc, 2 * jc + 1):
                            for sub in range(2):
                                isub = half * 2 + sub
                                for hh in range(2):
                                    nc.tensor.matmul(
                                        po[:, (sub * 2 + hh) * 65:
                                           (sub * 2 + hh + 1) * 65],
                                        ptiles[2 * jb + hh][
                                            :, isub * 128:(isub + 1) * 128],
                                        vt_sb[jb][:, hp * 2 + hh, :],
                                        start=False,
                                        stop=(jb == JB - 1),
                                        skip_group_check=True,
                                    )
                        if jc == 7:
                            # Normalization strategy: 1/l per (i, head) on
                            # DVE; a bf16 copy of po; and per-(isub, head) a
                            # diagonal matrix diag(1/l) built on GPSIMD
                            # (SBUF-only, so it is legal there). The finish
                            # pass then does transpose+normalize in one PE
                            # matmul: po_sb[:, dcols].T @ diag.
                            rcp = ovp.tile([128, 4], fp32, tag="rcp", name="rcp")
                            nc.vector.reciprocal(rcp[:], po[:, 64:260:65])
                            posb = ovp.tile([128, 260], bfl, tag="posb",
                                            name="posb")
                            if half == 0:
                                nc.scalar.copy(posb[:], po[:])
                            else:
                                nc.vector.tensor_copy(posb[:], po[:])
                            diags = []
                            for c in range(4):
                                diag = ovp.tile([128, 128], bfl, tag="diag",
                                                name="diag")
                                nc.gpsimd.tensor_scalar(
                                    diag[:], ident[:], rcp[:, c:c + 1],
                                    None, Mult,
                                )
                                diags.append(diag)
                            state["ohis"].append((posb, diags))
                    return f

                return [chunk(half, jc)
                        for half in range(2) for jc in range(8)], state

            def emit_pv_finish(it, hp, state_ohis):
                """Transposes + copies into oh_sb after the last po half."""
                if "pv" in skip or not state_ohis:
                    return
                for isub in range(4):
                    posb, diags = state_ohis[isub // 2]
                    sub = isub % 2
                    tp = wp.tile([128, 128], fp32, tag="w", name="tp")
                    for hh in range(2):
                        c = sub * 2 + hh
                        nc.tensor.matmul(
                            tp[hh * 64:(hh + 1) * 64, :],
                            posb[:, c * 65:c * 65 + 64],
                            diags[c][:],
                        )
                    nc.vector.tensor_copy(
                        oh_sb[hp][:, it * SLOT + isub * 128:
                                  it * SLOT + (isub + 1) * 128],
                        tp[:],
                    )

            def emit_outproj_tile(it, mt):
                yp = wp.tile([128, SLOT], fp32, tag="w", name="yp")
                for kt in range(2):
                    nc.tensor.matmul(
                        yp[:],
                        wo_sb[:, kt, mt * 128:(mt + 1) * 128],
                        oh_sb[kt][:, it * SLOT:(it + 1) * SLOT],
                        start=(kt == 0),
                        stop=(kt == 1),
                    )
                ys = ovp.tile([128, SLOT], bfl, tag="ys", name="ys")
                if mt % 2 == 0:
                    nc.vector.tensor_copy(ys[:], yp[:])
                else:
                    nc.scalar.copy(ys[:], yp[:])
                nc.sync.dma_start(
                    y_d[mt * 128:(mt + 1) * 128, it * SLOT:(it + 1) * SLOT],
                    ys[:],
                )

            def emit_outproj(it):
                if "pv" in skip:
                    return
                for mt in range(4):
                    emit_outproj_tile(it, mt)

            def emit_weave(it, hp, fillers, max_take=4):
                """One iteration: 32 (sim+exp) units with fillers drained
                between units, front-loaded just enough that none spill
                past the last unit."""
                ptiles = []
                n, fi = len(fillers), 0
                nu = 2 * JB
                for u in range(nu):
                    ptiles.append(emit_sim_unit(it, hp, u))
                    while fi < n and fi <= (u + 1) * n // nu - 1:
                        fillers[fi]()
                        fi += 1
                while fi < n:
                    fillers[fi]()
                    fi += 1
                return ptiles

            # ---- warmup: project what group 0 needs (q-hp0 i-tile 0 + all
            # of k-hp0), then weave group 0's sim+exp with the remaining
            # projection tiles as filler.
            emit_qkproj_tile(0, 0)
            for nt in range(NT):
                emit_qkproj_tile(1, nt)
            warm = (
                [lambda nt=nt: emit_qkproj_tile(0, nt) for nt in (1, 2, 3)]
                + [lambda nt=nt: emit_qkproj_tile(2, nt) for nt in range(NT)]
                + [lambda nt=nt: emit_qkproj_tile(3, nt) for nt in range(NT)]
                + [lambda jb=jb: emit_vtproj_tile(jb) for jb in range(JB)]
            )
            first = emit_weave(*groups[0], warm, max_take=2)

            # Steady state, two-deep pipeline: weave(k) = sim+exp(g_k) woven
            # with P@V(g_{k-1}); the transpose/copy finish and the output
            # projection of g_{k-2} run as the first fillers so their
            # normalize chain has had a full iteration to drain.
            prev = (groups[0][0], groups[0][1], first)
            fin = None
            for it, hp in groups[1:]:
                pit, php, pp_ = prev
                fills, st = make_pv_fillers(pit, php, pp_)
                pre = []
                if fin is not None:
                    fit, fhp, fohis = fin
                    pre.append(
                        lambda a=fit, b=fhp, c=fohis: emit_pv_finish(a, b, c))
                    if fhp == 1:
                        pre.append(lambda a=fit: (emit_outproj_tile(a, 0),
                                                  emit_outproj_tile(a, 1)))
                        pre.append(lambda a=fit: (emit_outproj_tile(a, 2),
                                                  emit_outproj_tile(a, 3)))
                ptiles = emit_weave(it, hp, pre + fills)
                fin = (pit, php, st["ohis"])
                prev = (it, hp, ptiles)

            # tail: last group's P@V with the previous finish woven in
            pit, php, pp_ = prev
            fills, st = make_pv_fillers(pit, php, pp_)
            fit, fhp, fohis = fin
            for i, f in enumerate(fills):
                f()
                if i == 2:
                    emit_pv_finish(fit, fhp, fohis)
                    if fhp == 1:
                        emit_outproj(fit)
            emit_pv_finish(pit, php, st["ohis"])
            emit_outproj(pit)

    nc.compile()
    return nc


def _get_program():
    global _PROG
    if _PROG is None:
        _PROG = _build_program()
    return _PROG


def _make_in_maps(x, w_qkv, w_out):
    xs = x.astype(bf16)
    ident = np.eye(128, dtype=bf16)
    in_maps = []
    for core in range(NCORES):
        b, g = divmod(core, 2)
        r0 = 256 * g
        wq = w_qkv[r0:r0 + 256, :] * SCALE
        wk = w_qkv[512 + r0:512 + r0 + 256, :]
        wv = w_qkv[1024 + r0:1024 + r0 + 256, :]
        wqkT = np.ascontiguousarray(
            np.concatenate(
                [wq[0:128], wk[0:128], wq[128:256], wk[128:256]], axis=0
            ).T.astype(bf16)
        )
        wvT = np.ascontiguousarray(wv.T.astype(bf16))
        woT = np.ascontiguousarray(w_out[:, r0:r0 + 256].T.astype(bf16))
        in_maps.append({
            "x": np.ascontiguousarray(xs[b]),
            "wqk": wqkT,
            "wv": wvT,
            "wo": woT,
            "ident": ident,
        })
    return in_maps


def _gather(results, b_out):
    parts = [np.asarray(results[c]["y"], dtype=np.float32)
             for c in range(NCORES)]
    y = np.stack([parts[2 * b] + parts[2 * b + 1] for b in range(B)])
    y += b_out[None, :, None]
    return y.astype(np.float32)


def kernel(x, w_qkv, w_out, b_out):
    from concourse.bass_utils import run_bass_kernel_spmd

    nc = _get_program()

    x = np.asarray(x, dtype=np.float32)
    w_qkv = np.asarray(w_qkv, dtype=np.float32)
    w_out = np.asarray(w_out, dtype=np.float32)
    b_out = np.asarray(b_out, dtype=np.float32)

    in_maps = _make_in_maps(x, w_qkv, w_out)
    res = run_bass_kernel_spmd(nc, in_maps, core_ids=list(range(NCORES)))
    return _gather(res.results, b_out)
